# revision 19
# baseline (speedup 1.0000x reference)
"""
Trainium2 Bass kernel v4 for nn_DKNN (differentiable kNN, NeuralSort + PL).

Math per (sample p, query m) pair (n=1024, K=16, tau=1):
    t = 2 q.nb - ||nb||^2 (softmax-equivalent to -||q-nb||^2)
    s = t + gumbel - mean              # centered (softmax-invariant)
    B_i = sum_j |s_i - s_j|            # O(n^2) hot loop
    l[r,i] = scaling_r * s_i - B_i
    out[i] = sum_r softmax_i(l[r,:])

v4 key identity:  B_i = 2*M_i - n*s_i - T,  M_i = sum_j max(s_i, s_j),
T = sum_j s_j.  M is ONE standard DVE tensor_scalar(op0=max,
op1=add-as-reduction, accum_out) per segment, which runs in 2x_2p
perf mode for fp32 SBUF operands (0.5 cyc/elem) -- 2.2x the custom
1x DVE op of v3.  The -n*s_i - T corrections fold into host constants:
   lhsg_s := scaling + n   (logits = (scaling+n)*s - 2*M; T drops out
   lhsg_b := -2             by softmax shift-invariance).
ACT units compute R'_i = sum_j relu(s_j - s_i) = M_i - n_seg*s_i and
are adjusted at the combine step with a +1024*s^T (pstK) tile.

Centering: s is centered per pair via an extra matmul column
(nbT[:,n] = sum_j nb_j so scores8[:,n] = sum_j 2q.nb_j) plus a host
-prescaled gumbel-sum column; this shrinks |scaling*s| products ~4x
and with them the dominant fp32 rounding error of v3.

Sharding: 64 (p,m) pairs -> 8 per core.
"""

import os
import sys

import numpy as np

sys.path.insert(0, "/opt/trn_rl_repo")


def _install_ntff_hook_shim():
    import types

    if "antenv.axon_hooks" in sys.modules:
        return
    mod = types.ModuleType("antenv.axon_hooks")
    state = {"hook": None}
    mod.set_axon_ntff_profile_hook = lambda h: state.__setitem__("hook", h)
    mod.get_axon_ntff_profile_hook = lambda: state["hook"]
    sys.modules["antenv.axon_hooks"] = mod
    try:
        from trn_agent_boot.trn_boot import _ntff_profile_via_ctypes

        mod.set_axon_ntff_profile_hook(
            _ntff_profile_via_ctypes("/opt/axon/libaxon_pjrt.so")
        )
    except Exception:
        pass


_install_ntff_hook_shim()

import concourse.bass as bass
import concourse.bass_isa as bass_isa
import concourse.mybir as mybir
import concourse.tile as tile
from concourse import bacc
from concourse.bass_utils import run_bass_kernel_spmd

F32 = mybir.dt.float32
F16 = mybir.dt.float16
AF = mybir.ActivationFunctionType
ALU = mybir.AluOpType
AX = mybir.AxisListType

N = 1024
N1 = N + 1          # scores matmul carries an extra sum column
D = 128
M = 32
S = 2
K = 16
NCORES = 8
PAIRS = 8
NCHUNK = 8
HALF = 512
QUART = 256
GROUPS = 2
GP = PAIRS // GROUPS

# chunk ownership (chunks 0..7 of 128 i-values each; every chunk is two
# 512-j segments).  ACT chunks use Relu convention (adjusted at combine),
# DVE/GPS chunks use max convention.
ACT_CHUNKS = tuple(int(c) for c in os.environ.get("DK_ACT_CHUNKS", "0123"))
GPS_SEGS = int(os.environ.get("DK_GPS_SEGS", "0"))  # segs of chunk 7 on GPS
ACT_FD = int(os.environ.get("DK_ACT_FD", "1024"))
DVE_FD = int(os.environ.get("DK_DVE_FD", "1024"))


def build_nc():
    nc = bacc.Bacc("TRN2", target_bir_lowering=False, debug=False)
    n_act = len(ACT_CHUNKS)
    assert ACT_CHUNKS == tuple(range(n_act)), "ACT chunks must be 0..k-1"

    with tile.TileContext(nc) as tc:
        with tc.tile_pool(name="dram", bufs=1, space="DRAM") as dram:
            d_nbT = dram.tile([D, N], F32, kind="ExternalInput", name="nbT", uniquify=False)
            d_nbs = dram.tile([D, 1], F32, kind="ExternalInput", name="nbs", uniquify=False)
            d_qT2 = dram.tile([D, PAIRS], F32, kind="ExternalInput", name="qT2", uniquify=False)
            d_gum8 = dram.tile([PAIRS, N1], F32, kind="ExternalInput", name="gum8", uniquify=False)
            d_ident = dram.tile([D, D], F32, kind="ExternalInput", name="ident", uniquify=False)
            d_lhsg_s = dram.tile([PAIRS, GROUPS * 16 * GP], F32, kind="ExternalInput", name="lhsg_s", uniquify=False)
            d_lhsg_b = dram.tile([GP, 16 * GP], F32, kind="ExternalInput", name="lhsg_b", uniquify=False)
            d_onesg = dram.tile([16 * GP, GP], F16, kind="ExternalInput", name="onesg", uniquify=False)
            d_out = dram.tile([PAIRS, N], F32, kind="ExternalOutput", name="out", uniquify=False)
            DK_DEBUG = bool(int(os.environ.get("DK_DEBUG", "0")))
            if DK_DEBUG:
                d_dbg_s = dram.tile([PAIRS, N], F32, kind="ExternalOutput", name="dbg_s", uniquify=False)
                d_dbg_m = dram.tile([GROUPS * GP, N], F32, kind="ExternalOutput", name="dbg_m", uniquify=False)

            with tc.tile_pool(name="consts", bufs=1) as consts:
                nbT = consts.tile([D, N], F32)
                nbs = consts.tile([D, 1], F32)
                qT2 = consts.tile([D, PAIRS], F32)
                gum8 = consts.tile([PAIRS, N1], F32)
                ident = consts.tile([D, D], F32)
                lhsg_s = consts.tile([PAIRS, GROUPS * 16 * GP], F32)
                lhsg_b = consts.tile([GP, 16 * GP], F32)
                onesg = consts.tile([16 * GP, GP], F16)
                warm = consts.tile([1, 16], F32)

                # qT2 first (gates the first scores matmul, tiny); nbT
                # slices spread over the three DMA issue paths so the
                # transfer isn't queue-bound; arrival rises with column
                # index (PE consumes quarters in order).
                nc.sync.dma_start(out=nbs[:], in_=d_nbs[:])
                nc.scalar.dma_start(out=qT2[:], in_=d_qT2[:])
                qeng = [nc.sync, nc.scalar, nc.sync,
                        nc.scalar, nc.gpsimd, nc.gpsimd]
                SL = 170
                for qd in range(6):
                    lo = qd * SL
                    hi = (qd + 1) * SL if qd < 5 else N
                    qeng[qd].dma_start(out=nbT[:, lo:hi], in_=d_nbT[:, lo:hi])
                nc.scalar.dma_start(out=gum8[:], in_=d_gum8[:])
                nc.sync.dma_start(out=ident[:], in_=d_ident[:])
                nc.vector.memset(warm[:], 0.0)

                # trigger the ACT table load early, off the critical path
                nc.scalar.activation(out=warm[:], in_=warm[:], func=AF.Abs,
                                     bias=0.0, scale=1.0)

                with tc.tile_pool(name="work", bufs=1) as work:
                    s_rows = work.tile([PAIRS, N], F32)
                    nm = work.tile([PAIRS, 1], F32)
                    srow = [work.tile([1, N], F32, name=f"srow{i}") for i in range(PAIRS)]
                    b_rows = [work.tile([GP, N], F32, name=f"br{g}") for g in range(GROUPS)]
                    pst = work.tile([D, PAIRS * NCHUNK], F32)    # col c*8+pr = +s_pr[128c+p]
                    nst = work.tile([D, PAIRS * NCHUNK], F32)    # -s^T
                    tcol8 = work.tile([D, PAIRS], F32)
                    tsum8 = work.tile([D, PAIRS], F32)   # col q = T_q on all partitions
                    b_seg = work.tile([D, 2 * PAIRS * NCHUNK], F32)  # col (8pr+c)*2+g
                    nc.gpsimd.memset(b_seg[:], 0.0)
                    zfull = work.tile([D, N], F32)
                    nc.gpsimd.memset(zfull[:], 0.0)
                    b_sum = work.tile([D, PAIRS * NCHUNK], F32)      # col 8pr+c = M_pr[128c+p]
                    tmp2 = work.tile([D, NCHUNK], F32)
                    bt_sb0 = [work.tile([4, D], F32, name=f"bta{p}") for p in range(PAIRS)]
                    bt_sb1 = [work.tile([4, D], F32, name=f"btb{p}") for p in range(PAIRS)]
                    e_sb = [work.tile([16 * GP, N], F16, name=f"e{g}") for g in range(GROUPS)]
                    p_sb = [work.tile([16 * GP, N], F16, name=f"p{g}") for g in range(GROUPS)]
                    negmax = [work.tile([16 * GP, 2], F32, name=f"nm{g}") for g in range(GROUPS)]
                    zden = [work.tile([16 * GP, 2], F32, name=f"z{g}") for g in range(GROUPS)]
                    invz = [work.tile([16 * GP, 1], F32, name=f"iz{g}") for g in range(GROUPS)]
                    nfm = [work.tile([16 * GP, 1], F32, name=f"nf{g}") for g in range(GROUPS)]
                    dmh = [work.tile([16 * GP, 2], F32, name=f"dm{g}") for g in range(GROUPS)]
                    fh = [work.tile([16 * GP, 2], F32, name=f"fh{g}") for g in range(GROUPS)]
                    zf = [work.tile([16 * GP, 2], F32, name=f"zf{g}") for g in range(GROUPS)]
                    zc = [work.tile([16 * GP, 1], F32, name=f"zc{g}") for g in range(GROUPS)]
                    sc2 = [work.tile([16 * GP, 2], F32, name=f"sc{g}") for g in range(GROUPS)]
                    out_sb = [work.tile([GP, N], F32, name=f"os{g}") for g in range(GROUPS)]

                    # ---- s = center((2 q.nb - nb2) + gumbel) ------------------
                    with tc.tile_pool(name="psum_s", bufs=1, space="PSUM") as pp_s:
                        scores8 = pp_s.tile([PAIRS, N], F32)
                        sc_ps = pp_s.tile([PAIRS, 1], F32)
                        # sum column first: tiny matmul against sum(nb) gives
                        # sum_j 2q.nb_j, the data part of the mean
                        nc.tensor.matmul(sc_ps[:], qT2[:], nbs[:],
                                         start=True, stop=True)
                        # negative mean: sc_ps*(-1/n) + gum8[:,N]
                        # (gum8[:,N] holds -sum(gum8)/n from the host)
                        nc.vector.scalar_tensor_tensor(
                            nm[:], sc_ps[:], -1.0 / N, gum8[:, N:N1],
                            op0=ALU.mult, op1=ALU.add)
                        for qd in range(4):
                            qs = slice(qd * QUART, (qd + 1) * QUART)
                            nc.tensor.matmul(scores8[:, qs], qT2[:], nbT[:, qs],
                                             start=True, stop=True)
                        # s_rows = (scores8 + nm) + gum8  (one fused pass)
                        nc.vector.scalar_tensor_tensor(
                            s_rows[:], scores8[:, 0:N], nm[:], gum8[:, 0:N],
                            op0=ALU.add, op1=ALU.add)
                        # stage rows on partition 0 for broadcasts
                        for pr in range(1, PAIRS):
                            nc.sync.dma_start(out=srow[pr][:],
                                              in_=s_rows[pr:pr + 1, :])
                        nc.sync.dma_start(out=lhsg_s[:], in_=d_lhsg_s[:])
                        nc.sync.dma_start(out=lhsg_b[:], in_=d_lhsg_b[:])
                        nc.sync.dma_start(out=onesg[:], in_=d_onesg[:])

                        # pst[p, c*8+pr] = s_pr[128c+p]; nst = -pst; pstK = n*pst
                        with tc.tile_pool(name="psum_st", bufs=1, space="PSUM") as pp_st:
                            st_ps = pp_st.tile([D, PAIRS * NCHUNK], F32)
                            for c in range(NCHUNK):
                                nc.tensor.transpose(
                                    st_ps[:, c * PAIRS:(c + 1) * PAIRS],
                                    s_rows[0:PAIRS, c * D:(c + 1) * D],
                                    ident[:PAIRS, :PAIRS],
                                )
                            nc.vector.tensor_copy(pst[:], st_ps[:])
                            nc.vector.tensor_scalar(nst[:], st_ps[:], -1.0,
                                                    None, ALU.mult)
                        # T_q = sum_j s_q[j], replicated on all partitions
                        nc.vector.tensor_reduce(
                            tcol8[:], pst[:].rearrange("p (c q) -> p q c", q=PAIRS),
                            AX.X, ALU.add)
                        nc.gpsimd.partition_all_reduce(
                            tsum8[:], tcol8[:], D, bass_isa.ReduceOp.add)

                    with tc.tile_pool(name="psum_l", bufs=1, space="PSUM") as pp_l, \
                         tc.tile_pool(name="psum_bt", bufs=1, space="PSUM") as pp_bt, \
                         tc.tile_pool(name="psum_bt2", bufs=1, space="PSUM") as pp_bt2, \
                         tc.tile_pool(name="psum_o", bufs=1, space="PSUM") as pp_o:
                        logits = [pp_l.tile([16 * GP, N], F32, name=f"lg{g}")
                                  for g in range(GROUPS)]
                        # s-part of logits: early matmuls (open accumulation)
                        for g in range(GROUPS):
                            for h in range(2):
                                hs = slice(h * HALF, (h + 1) * HALF)
                                nc.tensor.matmul(
                                    logits[g][:, hs],
                                    lhsg_s[:, 16 * GP * g:16 * GP * (g + 1)],
                                    s_rows[:, hs],
                                    start=True, stop=False)

                        # ---- B phase: M_i = sum_j max(s_i, s_j) -------------
                        def _seg_dve(pr, c, gseg, fd=HALF):
                            # STT (modes=[]) keeps the scheduler's cost model
                            # honest: plain tensor_scalar is modeled at 2x_2p
                            # but runs 1x on HW, which skews the schedule.
                            base2 = (pr * NCHUNK) * 2
                            bsl = bcast[:, gseg * fd:(gseg + 1) * fd]
                            scr = scr_dve.tile([D, fd], F32, tag=f"sv{fd}")
                            nc.vector.scalar_tensor_tensor(
                                scr[:], bsl,
                                pst[:, c * PAIRS + pr: c * PAIRS + pr + 1],
                                zfull[:, 0:fd], op0=ALU.min, op1=ALU.bypass,
                                accum_out=b_seg[:, base2 + c * 2 + gseg:
                                                base2 + c * 2 + gseg + 1])

                        def _seg_gps(pr, c, gseg):
                            base2 = (pr * NCHUNK) * 2
                            scr = scr_gps.tile([D, HALF], F32, tag="sg")
                            nc.gpsimd.tensor_scalar(
                                scr[:],
                                bcast_of[pr][:, gseg * HALF:(gseg + 1) * HALF],
                                pst[:, c * PAIRS + pr: c * PAIRS + pr + 1],
                                None, ALU.min, op1=ALU.add,
                                accum_out=b_seg[:, base2 + c * 2 + gseg:
                                                base2 + c * 2 + gseg + 1])

                        def _seg_act(pr, c, gseg, fd):
                            # R' = sum relu(s_j - s_i); combine adds n*s_i
                            base2 = (pr * NCHUNK) * 2
                            scr = scr_act.tile([D, fd], F32, tag="sa")
                            nc.scalar.activation(
                                out=scr[:],
                                in_=bcast[:, gseg * fd:(gseg + 1) * fd],
                                func=AF.Relu,
                                bias=nst[:, c * PAIRS + pr: c * PAIRS + pr + 1],
                                scale=1.0,
                                accum_out=b_seg[:, base2 + c * 2 + gseg:
                                                base2 + c * 2 + gseg + 1],
                            )

                        def emit_units(pr):
                            for c in ACT_CHUNKS:
                                if ACT_FD == 1024:
                                    _seg_act(pr, c, 0, 1024)
                                else:
                                    _seg_act(pr, c, 0, HALF)
                                    _seg_act(pr, c, 1, HALF)
                            for c in range(n_act, NCHUNK):
                                if c == NCHUNK - 1 and GPS_SEGS >= 2:
                                    continue  # GPS-owned
                                if DVE_FD == 1024:
                                    _seg_dve(pr, c, 0, 1024)
                                else:
                                    _seg_dve(pr, c, 0)
                                    if not (c == NCHUNK - 1 and GPS_SEGS == 1):
                                        _seg_dve(pr, c, 1)

                        def emit_pair_brow(pr):
                            # per i-half: combine -> transpose -> copy -> DMA,
                            # so each b_row half unblocks its half of the
                            # group tail as soon as its engine finishes.
                            g, q = pr // GP, pr % GP
                            sl16 = slice(pr * NCHUNK * 2, (pr + 1) * NCHUNK * 2)
                            segs = b_seg[:, sl16].rearrange("p (u g) -> p u g", g=2)
                            # ACT chunks (i-half 0): Lo = T - (seg0+seg1)
                            nc.vector.scalar_tensor_tensor(
                                tmp2[:, 0:n_act], segs[:, 0:n_act, 0], -1.0,
                                segs[:, 0:n_act, 1],
                                op0=ALU.mult, op1=ALU.subtract)
                            nc.vector.tensor_scalar(
                                b_sum[:, pr * NCHUNK: pr * NCHUNK + n_act],
                                tmp2[:, 0:n_act], tsum8[:, pr:pr + 1],
                                None, ALU.add)
                            bt_ps0 = pp_bt.tile([4, D], F32, tag="bt0")
                            nc.tensor.transpose(
                                bt_ps0[:], b_sum[:, pr * NCHUNK: pr * NCHUNK + 4],
                                ident[:])
                            nc.scalar.copy(bt_sb0[pr][:], bt_ps0[:])
                            nc.sync.dma_start(out=b_rows[g][q:q + 1, 0:HALF],
                                              in_=bt_sb0[pr][:])
                            # DVE/GPS chunks (i-half 1): plain seg sum
                            nc.vector.tensor_reduce(
                                b_sum[:, pr * NCHUNK + n_act: (pr + 1) * NCHUNK],
                                segs[:, n_act:NCHUNK, :], AX.X, ALU.add)
                            bt_ps1 = pp_bt2.tile([4, D], F32, tag="bt1")
                            nc.tensor.transpose(
                                bt_ps1[:], b_sum[:, pr * NCHUNK + 4:(pr + 1) * NCHUNK],
                                ident[:])
                            nc.vector.tensor_copy(bt_sb1[pr][:], bt_ps1[:])
                            nc.sync.dma_start(out=b_rows[g][q:q + 1, HALF:N],
                                              in_=bt_sb1[pr][:])

                        def emit_group_tail(g):
                            # flash softmax over the two i-halves: half-0 work
                            # (logits mm, max, exp) overlaps half-1's B phase.
                            for h in range(2):
                                hs = slice(h * HALF, (h + 1) * HALF)
                                nc.tensor.matmul(
                                    logits[g][:, hs], lhsg_b[:],
                                    b_rows[g][:, hs],
                                    start=False, stop=True)
                                nc.vector.tensor_reduce(
                                    negmax[g][:, h:h + 1], logits[g][:, hs],
                                    AX.X, ALU.max, negate=True)
                                nc.scalar.activation(
                                    out=e_sb[g][:, hs], in_=logits[g][:, hs],
                                    func=AF.Exp, bias=negmax[g][:, h:h + 1],
                                    scale=1.0, accum_out=zden[g][:, h:h + 1])
                            # combine stats: nfm = -m = min_h(-m_h)
                            nc.vector.tensor_reduce(nfm[g][:], negmax[g][:, 0:2],
                                                    AX.X, ALU.min)
                            # fh = e^{m_h - m} = exp(-((-m_h) - (-m)))
                            nc.vector.tensor_scalar(dmh[g][:], negmax[g][:, 0:2],
                                                    nfm[g][:], None, ALU.subtract)
                            nc.scalar.activation(out=fh[g][:], in_=dmh[g][:],
                                                 func=AF.Exp, bias=0.0, scale=-1.0)
                            nc.vector.tensor_tensor(zf[g][:], zden[g][:, 0:2],
                                                    fh[g][:], ALU.mult)
                            nc.vector.tensor_reduce(zc[g][:], zf[g][:], AX.X, ALU.add)
                            nc.vector.reciprocal(invz[g][:], zc[g][:])
                            nc.vector.tensor_scalar(sc2[g][:], fh[g][:],
                                                    invz[g][:], None, ALU.mult)
                            out_ps = pp_o.tile([GP, N], F32, tag="op")
                            for h in range(2):
                                hs = slice(h * HALF, (h + 1) * HALF)
                                nc.vector.tensor_scalar(
                                    p_sb[g][:, hs], e_sb[g][:, hs],
                                    sc2[g][:, h:h + 1], None, ALU.mult)
                                nc.tensor.matmul(out_ps[:, hs], onesg[:],
                                                 p_sb[g][:, hs], start=True, stop=True)
                            return out_ps

                        def emit_group_finish(g, out_ps):
                            nc.scalar.copy(out_sb[g][:, 0:HALF], out_ps[:, 0:HALF])
                            nc.vector.tensor_copy(out_sb[g][:, HALF:N],
                                                  out_ps[:, HALF:N])
                            nc.sync.dma_start(out=d_out[GP * g:GP * (g + 1), 0:HALF],
                                              in_=out_sb[g][:, 0:HALF])
                            nc.scalar.dma_start(out=d_out[GP * g:GP * (g + 1), HALF:N],
                                                in_=out_sb[g][:, HALF:N])

                        with tc.tile_pool(name="bcast", bufs=3) as bc_pool, \
                             tc.tile_pool(name="scr_act", bufs=6) as scr_act, \
                             tc.tile_pool(name="scr_dve", bufs=6) as scr_dve, \
                             tc.tile_pool(name="scr_gps", bufs=4) as scr_gps:
                            # GPS broadcasts run two pairs ahead of the GPS
                            # B-segs; per-pair consumers are emitted lag-2 so
                            # emission order matches data order on every engine.
                            bcast_of = {}
                            LAG = 2 if GPS_SEGS else 0
                            for it in range(PAIRS + LAG):
                                if it < PAIRS:
                                    bc = bc_pool.tile([D, N], F32, tag="bcast")
                                    bcast_of[it] = bc
                                    src = s_rows[0:1, :] if it == 0 else srow[it][:]
                                    nc.gpsimd.partition_broadcast(bc[:], src)
                                pr = it - LAG
                                if pr < 0:
                                    continue
                                bcast = bcast_of[pr]
                                for gseg in range(2 - GPS_SEGS, 2):
                                    _seg_gps(pr, NCHUNK - 1, gseg)
                                emit_units(pr)
                                emit_pair_brow(pr)
                                if pr == GP - 1:
                                    ops0 = emit_group_tail(0)
                            emit_group_finish(0, ops0)
                            ops1 = emit_group_tail(1)
                            emit_group_finish(1, ops1)
                            if DK_DEBUG:
                                nc.sync.dma_start(out=d_dbg_s[:], in_=s_rows[:])
                                for g in range(GROUPS):
                                    nc.sync.dma_start(
                                        out=d_dbg_m[GP * g:GP * (g + 1), :],
                                        in_=b_rows[g][:])

    nc.finalize()
    return nc


def host_inputs(query, neighbors, gumbel):
    """Per-core input maps. Core c handles pairs [8c, 8c+8)."""
    query = np.asarray(query, np.float32)
    neighbors = np.asarray(neighbors, np.float32)
    gumbel = np.asarray(gumbel, np.float32)

    nbT = np.ascontiguousarray(neighbors.T)
    nbs = np.ascontiguousarray(nbT.sum(1, keepdims=True))
    nb2 = np.sum(neighbors * neighbors, 1)[None, :]
    ident = np.eye(D, dtype=np.float32)

    # logits = (scaling - n) * s + 2*Lo   (see header derivation)
    scaling = (N + 1 - 2 * np.arange(1, K + 1)).astype(np.float32) - float(N)
    lhsg_s = np.zeros((PAIRS, GROUPS * 16 * GP), np.float32)
    lhsg_b = np.zeros((GP, 16 * GP), np.float32)
    onesg = np.zeros((16 * GP, GP), np.float16)
    for q in range(GP):
        for g in range(GROUPS):
            lhsg_s[GP * g + q, 16 * GP * g + 16 * q:16 * GP * g + 16 * q + K] = scaling
        lhsg_b[q, 16 * q:16 * q + K] = 2.0
        onesg[16 * q:16 * q + K, q] = 1.0

    gflat = gumbel.reshape(S * M, N)
    in_maps = []
    for c in range(NCORES):
        m0 = (PAIRS * c) % M
        g8 = np.ascontiguousarray(gflat[PAIRS * c:PAIRS * (c + 1)] - nb2)
        g8_aug = np.concatenate(
            [g8, (-g8.sum(1, keepdims=True) / N).astype(np.float32)], 1)
        in_maps.append({
            "nbT": nbT,
            "nbs": nbs,
            "qT2": np.ascontiguousarray(2.0 * query.T[:, m0:m0 + PAIRS]),
            "gum8": g8_aug,
            "ident": ident,
            "lhsg_s": lhsg_s,
            "lhsg_b": lhsg_b,
            "onesg": onesg,
        })
    return in_maps


_NC_CACHE = {}


def _get_nc():
    if "nc" not in _NC_CACHE:
        _NC_CACHE["nc"] = build_nc()
    return _NC_CACHE["nc"]


def run(query, neighbors, gumbel, trace=False):
    nc = _get_nc()
    in_maps = host_inputs(query, neighbors, gumbel)
    res = run_bass_kernel_spmd(nc, in_maps, list(range(NCORES)), trace=trace)
    outs = np.stack([res.results[c]["out"] for c in range(NCORES)])
    full = outs.reshape(S, M, N).astype(np.float32)
    return full, res


def kernel(query, neighbors, gumbel):
    full, _ = run(query, neighbors, gumbel, trace=False)
    return full


def _numpy_model(query, neighbors, gumbel):
    q = np.asarray(query, np.float32)
    nb = np.asarray(neighbors, np.float32)
    g = np.asarray(gumbel, np.float32).reshape(S * M, N)
    t = 2.0 * q @ nb.T - np.sum(nb * nb, 1)[None, :]
    t = np.concatenate([t, t], 0)
    s = t + g
    s = s - s.mean(1, keepdims=True)
    B = np.abs(s[:, :, None] - s[:, None, :]).sum(2)
    scaling = (N + 1 - 2 * np.arange(1, K + 1)).astype(np.float32)
    l = scaling[None, :, None] * s[:, None, :] - B[:, None, :]
    l = l - l.max(2, keepdims=True)
    e = np.exp(l)
    p = e / e.sum(2, keepdims=True)
    return p.sum(1).reshape(S, M, N)


def _selftest_sim():
    from concourse.bass_interp import CoreSim

    rng = np.random.default_rng(0)
    query = rng.normal(size=(M, D)).astype(np.float32)
    neighbors = rng.normal(size=(N, D)).astype(np.float32)
    u = rng.uniform(1e-6, 1 - 1e-6, size=(S, M, N)).astype(np.float32)
    gumbel = -np.log(-np.log(u)).astype(np.float32)

    nc = _get_nc()
    in_maps = host_inputs(query, neighbors, gumbel)
    sim = CoreSim(nc)
    for k, v in in_maps[0].items():
        sim.tensor(k)[:] = v
    sim.simulate()
    got = np.array(sim.tensor("out"))
    want = _numpy_model(query, neighbors, gumbel).reshape(S * M, N)[:PAIRS]
    err = np.linalg.norm(got - want) / np.linalg.norm(want)
    print("sim rel err:", err)
    print("sim time (model ns):", sim.time)
    assert err < 2e-2, err
    print("SIM PASS")


if __name__ == "__main__":
    if "--sim" in sys.argv:
        _selftest_sim()


# revision 20
# speedup vs baseline: 1.0116x; 1.0116x over previous
"""
Trainium2 Bass kernel v4 for nn_DKNN (differentiable kNN, NeuralSort + PL).

Math per (sample p, query m) pair (n=1024, K=16, tau=1):
    t = 2 q.nb - ||nb||^2 (softmax-equivalent to -||q-nb||^2)
    s = t + gumbel - mean              # centered (softmax-invariant)
    B_i = sum_j |s_i - s_j|            # O(n^2) hot loop
    l[r,i] = scaling_r * s_i - B_i
    out[i] = sum_r softmax_i(l[r,:])

v4 key identity:  B_i = 2*M_i - n*s_i - T,  M_i = sum_j max(s_i, s_j),
T = sum_j s_j.  M is ONE standard DVE tensor_scalar(op0=max,
op1=add-as-reduction, accum_out) per segment, which runs in 2x_2p
perf mode for fp32 SBUF operands (0.5 cyc/elem) -- 2.2x the custom
1x DVE op of v3.  The -n*s_i - T corrections fold into host constants:
   lhsg_s := scaling + n   (logits = (scaling+n)*s - 2*M; T drops out
   lhsg_b := -2             by softmax shift-invariance).
ACT units compute R'_i = sum_j relu(s_j - s_i) = M_i - n_seg*s_i and
are adjusted at the combine step with a +1024*s^T (pstK) tile.

Centering: s is centered per pair via an extra matmul column
(nbT[:,n] = sum_j nb_j so scores8[:,n] = sum_j 2q.nb_j) plus a host
-prescaled gumbel-sum column; this shrinks |scaling*s| products ~4x
and with them the dominant fp32 rounding error of v3.

Sharding: 64 (p,m) pairs -> 8 per core.
"""

import os
import sys

import numpy as np

sys.path.insert(0, "/opt/trn_rl_repo")


def _install_ntff_hook_shim():
    import types

    if "antenv.axon_hooks" in sys.modules:
        return
    mod = types.ModuleType("antenv.axon_hooks")
    state = {"hook": None}
    mod.set_axon_ntff_profile_hook = lambda h: state.__setitem__("hook", h)
    mod.get_axon_ntff_profile_hook = lambda: state["hook"]
    sys.modules["antenv.axon_hooks"] = mod
    try:
        from trn_agent_boot.trn_boot import _ntff_profile_via_ctypes

        mod.set_axon_ntff_profile_hook(
            _ntff_profile_via_ctypes("/opt/axon/libaxon_pjrt.so")
        )
    except Exception:
        pass


_install_ntff_hook_shim()

import concourse.bass as bass
import concourse.bass_isa as bass_isa
import concourse.mybir as mybir
import concourse.tile as tile
from concourse import bacc
from concourse.bass_utils import run_bass_kernel_spmd

F32 = mybir.dt.float32
F16 = mybir.dt.float16
AF = mybir.ActivationFunctionType
ALU = mybir.AluOpType
AX = mybir.AxisListType

N = 1024
N1 = N + 1          # scores matmul carries an extra sum column
D = 128
M = 32
S = 2
K = 16
NCORES = 8
PAIRS = 8
NCHUNK = 8
HALF = 512
QUART = 256
GROUPS = 2
GP = PAIRS // GROUPS

# chunk ownership (chunks 0..7 of 128 i-values each; every chunk is two
# 512-j segments).  ACT chunks use Relu convention (adjusted at combine),
# DVE/GPS chunks use max convention.
ACT_CHUNKS = tuple(int(c) for c in os.environ.get("DK_ACT_CHUNKS", "0123"))
GPS_SEGS = int(os.environ.get("DK_GPS_SEGS", "0"))  # segs of chunk 7 on GPS
ACT_FD = int(os.environ.get("DK_ACT_FD", "1024"))
DVE_FD = int(os.environ.get("DK_DVE_FD", "1024"))


def build_nc():
    nc = bacc.Bacc("TRN2", target_bir_lowering=False, debug=False)
    n_act = len(ACT_CHUNKS)
    assert ACT_CHUNKS == tuple(range(n_act)), "ACT chunks must be 0..k-1"

    with tile.TileContext(nc) as tc:
        with tc.tile_pool(name="dram", bufs=1, space="DRAM") as dram:
            d_nbT = dram.tile([D, N], F32, kind="ExternalInput", name="nbT", uniquify=False)
            d_nbs = dram.tile([D, 1], F32, kind="ExternalInput", name="nbs", uniquify=False)
            d_qT2 = dram.tile([D, PAIRS], F32, kind="ExternalInput", name="qT2", uniquify=False)
            d_gum8 = dram.tile([PAIRS, N1], F32, kind="ExternalInput", name="gum8", uniquify=False)
            d_ident = dram.tile([D, D], F32, kind="ExternalInput", name="ident", uniquify=False)
            d_lhsg_s = dram.tile([PAIRS, GROUPS * 16 * GP], F32, kind="ExternalInput", name="lhsg_s", uniquify=False)
            d_lhsg_b = dram.tile([GP, 16 * GP], F32, kind="ExternalInput", name="lhsg_b", uniquify=False)
            d_onesg = dram.tile([16 * GP, GP], F16, kind="ExternalInput", name="onesg", uniquify=False)
            d_out = dram.tile([PAIRS, N], F32, kind="ExternalOutput", name="out", uniquify=False)
            DK_DEBUG = bool(int(os.environ.get("DK_DEBUG", "0")))
            if DK_DEBUG:
                d_dbg_s = dram.tile([PAIRS, N], F32, kind="ExternalOutput", name="dbg_s", uniquify=False)
                d_dbg_m = dram.tile([GROUPS * GP, N], F32, kind="ExternalOutput", name="dbg_m", uniquify=False)

            with tc.tile_pool(name="consts", bufs=1) as consts:
                nbT = consts.tile([D, N], F32)
                nbs = consts.tile([D, 1], F32)
                qT2 = consts.tile([D, PAIRS], F32)
                gum8 = consts.tile([PAIRS, N1], F32)
                ident = consts.tile([D, D], F32)
                lhsg_s = consts.tile([PAIRS, GROUPS * 16 * GP], F32)
                lhsg_b = consts.tile([GP, 16 * GP], F32)
                onesg = consts.tile([16 * GP, GP], F16)
                warm = consts.tile([1, 16], F32)

                # qT2 first (gates the first scores matmul, tiny); nbT
                # slices spread over the three DMA issue paths so the
                # transfer isn't queue-bound; arrival rises with column
                # index (PE consumes quarters in order).
                nc.sync.dma_start(out=nbs[:], in_=d_nbs[:])
                nc.scalar.dma_start(out=qT2[:], in_=d_qT2[:])
                qeng = [nc.sync, nc.scalar, nc.sync,
                        nc.scalar, nc.gpsimd, nc.gpsimd]
                SL = 170
                for qd in range(6):
                    lo = qd * SL
                    hi = (qd + 1) * SL if qd < 5 else N
                    qeng[qd].dma_start(out=nbT[:, lo:hi], in_=d_nbT[:, lo:hi])
                nc.scalar.dma_start(out=gum8[:], in_=d_gum8[:])
                nc.sync.dma_start(out=ident[:], in_=d_ident[:])
                nc.vector.memset(warm[:], 0.0)

                # trigger the ACT table load early, off the critical path
                nc.scalar.activation(out=warm[:], in_=warm[:], func=AF.Abs,
                                     bias=0.0, scale=1.0)

                with tc.tile_pool(name="work", bufs=1) as work:
                    s_rows = work.tile([PAIRS, N], F32)
                    nm = work.tile([PAIRS, 1], F32)
                    srow = [work.tile([1, N], F32, name=f"srow{i}") for i in range(PAIRS)]
                    b_rows = [work.tile([GP, N], F32, name=f"br{g}") for g in range(GROUPS)]
                    pst = work.tile([D, PAIRS * NCHUNK], F32)    # col c*8+pr = +s_pr[128c+p]
                    nst = work.tile([D, PAIRS * NCHUNK], F32)    # -s^T
                    tcol8 = work.tile([D, PAIRS], F32)
                    tsum8 = work.tile([D, PAIRS], F32)   # col q = T_q on all partitions
                    b_seg = work.tile([D, 2 * PAIRS * NCHUNK], F32)  # col (8pr+c)*2+g
                    nc.gpsimd.memset(b_seg[:], 0.0)
                    zfull = work.tile([D, N], F32)
                    nc.gpsimd.memset(zfull[:], 0.0)
                    b_sum = work.tile([D, PAIRS * NCHUNK], F32)      # col 8pr+c = M_pr[128c+p]
                    tmp2 = work.tile([D, NCHUNK], F32)
                    bt_sb0 = [work.tile([4, D], F32, name=f"bta{p}") for p in range(PAIRS)]
                    bt_sb1 = [work.tile([4, D], F32, name=f"btb{p}") for p in range(PAIRS)]
                    e_sb = [work.tile([16 * GP, N], F16, name=f"e{g}") for g in range(GROUPS)]
                    p_sb = [work.tile([16 * GP, N], F16, name=f"p{g}") for g in range(GROUPS)]
                    negmax = [work.tile([16 * GP, 2], F32, name=f"nm{g}") for g in range(GROUPS)]
                    zden = [work.tile([16 * GP, 2], F32, name=f"z{g}") for g in range(GROUPS)]
                    invz = [work.tile([16 * GP, 1], F32, name=f"iz{g}") for g in range(GROUPS)]
                    nfm = [work.tile([16 * GP, 1], F32, name=f"nf{g}") for g in range(GROUPS)]
                    dmh = [work.tile([16 * GP, 2], F32, name=f"dm{g}") for g in range(GROUPS)]
                    fh = [work.tile([16 * GP, 2], F32, name=f"fh{g}") for g in range(GROUPS)]
                    zf = [work.tile([16 * GP, 2], F32, name=f"zf{g}") for g in range(GROUPS)]
                    zc = [work.tile([16 * GP, 1], F32, name=f"zc{g}") for g in range(GROUPS)]
                    sc2 = [work.tile([16 * GP, 2], F32, name=f"sc{g}") for g in range(GROUPS)]
                    out_sb = [work.tile([GP, N], F32, name=f"os{g}") for g in range(GROUPS)]

                    # ---- s = center((2 q.nb - nb2) + gumbel) ------------------
                    with tc.tile_pool(name="psum_s", bufs=1, space="PSUM") as pp_s:
                        scores8 = pp_s.tile([PAIRS, N], F32)
                        sc_ps = pp_s.tile([PAIRS, 1], F32)
                        # sum column first: tiny matmul against sum(nb) gives
                        # sum_j 2q.nb_j, the data part of the mean
                        nc.tensor.matmul(sc_ps[:], qT2[:], nbs[:],
                                         start=True, stop=True)
                        # negative mean: sc_ps*(-1/n) + gum8[:,N]
                        # (gum8[:,N] holds -sum(gum8)/n from the host)
                        nc.vector.scalar_tensor_tensor(
                            nm[:], sc_ps[:], -1.0 / N, gum8[:, N:N1],
                            op0=ALU.mult, op1=ALU.add)
                        for qd in range(4):
                            qs = slice(qd * QUART, (qd + 1) * QUART)
                            nc.tensor.matmul(scores8[:, qs], qT2[:], nbT[:, qs],
                                             start=True, stop=True)
                        # s_rows = (scores8 + nm) + gum8  (one fused pass)
                        nc.vector.scalar_tensor_tensor(
                            s_rows[:], scores8[:, 0:N], nm[:], gum8[:, 0:N],
                            op0=ALU.add, op1=ALU.add)
                        # stage rows on partition 0 for broadcasts
                        for pr in range(1, PAIRS):
                            nc.sync.dma_start(out=srow[pr][:],
                                              in_=s_rows[pr:pr + 1, :])
                        nc.sync.dma_start(out=lhsg_s[:], in_=d_lhsg_s[:])
                        nc.sync.dma_start(out=lhsg_b[:], in_=d_lhsg_b[:])
                        nc.sync.dma_start(out=onesg[:], in_=d_onesg[:])

                        # pst[p, c*8+pr] = s_pr[128c+p]; nst = -pst; pstK = n*pst
                        with tc.tile_pool(name="psum_st", bufs=1, space="PSUM") as pp_st:
                            st_ps = pp_st.tile([D, PAIRS * NCHUNK], F32)
                            for c in range(NCHUNK):
                                nc.tensor.transpose(
                                    st_ps[:, c * PAIRS:(c + 1) * PAIRS],
                                    s_rows[0:PAIRS, c * D:(c + 1) * D],
                                    ident[:PAIRS, :PAIRS],
                                )
                            nc.vector.tensor_copy(pst[:], st_ps[:])
                            nc.vector.tensor_scalar(nst[:], st_ps[:], -1.0,
                                                    None, ALU.mult)
                        # T_q = sum_j s_q[j], replicated on all partitions
                        nc.vector.tensor_reduce(
                            tcol8[:], pst[:].rearrange("p (c q) -> p q c", q=PAIRS),
                            AX.X, ALU.add)
                        nc.gpsimd.partition_all_reduce(
                            tsum8[:], tcol8[:], D, bass_isa.ReduceOp.add)

                    with tc.tile_pool(name="psum_l", bufs=1, space="PSUM") as pp_l, \
                         tc.tile_pool(name="psum_bt", bufs=1, space="PSUM") as pp_bt, \
                         tc.tile_pool(name="psum_bt2", bufs=1, space="PSUM") as pp_bt2, \
                         tc.tile_pool(name="psum_o", bufs=1, space="PSUM") as pp_o:
                        logits = [pp_l.tile([16 * GP, N], F32, name=f"lg{g}")
                                  for g in range(GROUPS)]
                        # s-part of logits: early matmuls (open accumulation)
                        for g in range(GROUPS):
                            for h in range(2):
                                hs = slice(h * HALF, (h + 1) * HALF)
                                nc.tensor.matmul(
                                    logits[g][:, hs],
                                    lhsg_s[:, 16 * GP * g:16 * GP * (g + 1)],
                                    s_rows[:, hs],
                                    start=True, stop=False)

                        # ---- B phase: M_i = sum_j max(s_i, s_j) -------------
                        def _seg_dve(pr, c, gseg, fd=HALF):
                            # STT (modes=[]) keeps the scheduler's cost model
                            # honest: plain tensor_scalar is modeled at 2x_2p
                            # but runs 1x on HW, which skews the schedule.
                            base2 = (pr * NCHUNK) * 2
                            bsl = bcast[:, gseg * fd:(gseg + 1) * fd]
                            scr = scr_dve.tile([D, fd], F32, tag=f"sv{fd}")
                            nc.vector.scalar_tensor_tensor(
                                scr[:], bsl,
                                pst[:, c * PAIRS + pr: c * PAIRS + pr + 1],
                                zfull[:, 0:fd], op0=ALU.min, op1=ALU.bypass,
                                accum_out=b_seg[:, base2 + c * 2 + gseg:
                                                base2 + c * 2 + gseg + 1])

                        def _seg_gps(pr, c, gseg):
                            base2 = (pr * NCHUNK) * 2
                            scr = scr_gps.tile([D, HALF], F32, tag="sg")
                            nc.gpsimd.tensor_scalar(
                                scr[:],
                                bcast_of[pr][:, gseg * HALF:(gseg + 1) * HALF],
                                pst[:, c * PAIRS + pr: c * PAIRS + pr + 1],
                                None, ALU.min, op1=ALU.add,
                                accum_out=b_seg[:, base2 + c * 2 + gseg:
                                                base2 + c * 2 + gseg + 1])

                        def _seg_act(pr, c, gseg, fd):
                            # R' = sum relu(s_j - s_i); combine adds n*s_i
                            base2 = (pr * NCHUNK) * 2
                            scr = scr_act.tile([D, fd], F32, tag="sa")
                            nc.scalar.activation(
                                out=scr[:],
                                in_=bcast[:, gseg * fd:(gseg + 1) * fd],
                                func=AF.Relu,
                                bias=nst[:, c * PAIRS + pr: c * PAIRS + pr + 1],
                                scale=1.0,
                                accum_out=b_seg[:, base2 + c * 2 + gseg:
                                                base2 + c * 2 + gseg + 1],
                            )

                        def emit_units(pr):
                            for c in ACT_CHUNKS:
                                if ACT_FD == 1024:
                                    _seg_act(pr, c, 0, 1024)
                                else:
                                    _seg_act(pr, c, 0, HALF)
                                    _seg_act(pr, c, 1, HALF)
                            for c in range(n_act, NCHUNK):
                                if c == NCHUNK - 1 and GPS_SEGS >= 2:
                                    continue  # GPS-owned
                                if DVE_FD == 1024:
                                    _seg_dve(pr, c, 0, 1024)
                                else:
                                    _seg_dve(pr, c, 0)
                                    if not (c == NCHUNK - 1 and GPS_SEGS == 1):
                                        _seg_dve(pr, c, 1)

                        def emit_pair_brow(pr):
                            # per i-half: combine -> transpose -> copy -> DMA,
                            # so each b_row half unblocks its half of the
                            # group tail as soon as its engine finishes.
                            g, q = pr // GP, pr % GP
                            sl16 = slice(pr * NCHUNK * 2, (pr + 1) * NCHUNK * 2)
                            segs = b_seg[:, sl16].rearrange("p (u g) -> p u g", g=2)
                            # ACT chunks (i-half 0): Lo = T - (seg0+seg1)
                            nc.vector.scalar_tensor_tensor(
                                tmp2[:, 0:n_act], segs[:, 0:n_act, 0], -1.0,
                                segs[:, 0:n_act, 1],
                                op0=ALU.mult, op1=ALU.subtract)
                            nc.vector.tensor_scalar(
                                b_sum[:, pr * NCHUNK: pr * NCHUNK + n_act],
                                tmp2[:, 0:n_act], tsum8[:, pr:pr + 1],
                                None, ALU.add)
                            bt_ps0 = pp_bt.tile([4, D], F32, tag="bt0")
                            nc.tensor.transpose(
                                bt_ps0[:], b_sum[:, pr * NCHUNK: pr * NCHUNK + 4],
                                ident[:])
                            nc.vector.tensor_copy(bt_sb0[pr][:], bt_ps0[:])
                            nc.sync.dma_start(out=b_rows[g][q:q + 1, 0:HALF],
                                              in_=bt_sb0[pr][:])
                            # DVE/GPS chunks (i-half 1): plain seg sum
                            nc.vector.tensor_reduce(
                                b_sum[:, pr * NCHUNK + n_act: (pr + 1) * NCHUNK],
                                segs[:, n_act:NCHUNK, :], AX.X, ALU.add)
                            bt_ps1 = pp_bt2.tile([4, D], F32, tag="bt1")
                            nc.tensor.transpose(
                                bt_ps1[:], b_sum[:, pr * NCHUNK + 4:(pr + 1) * NCHUNK],
                                ident[:])
                            nc.vector.tensor_copy(bt_sb1[pr][:], bt_ps1[:])
                            nc.sync.dma_start(out=b_rows[g][q:q + 1, HALF:N],
                                              in_=bt_sb1[pr][:])

                        def emit_group_tail(g):
                            # flash softmax over the two i-halves: half-0 work
                            # (logits mm, max, exp) overlaps half-1's B phase.
                            for h in range(2):
                                hs = slice(h * HALF, (h + 1) * HALF)
                                nc.tensor.matmul(
                                    logits[g][:, hs], lhsg_b[:],
                                    b_rows[g][:, hs],
                                    start=False, stop=True)
                                nc.vector.tensor_reduce(
                                    negmax[g][:, h:h + 1], logits[g][:, hs],
                                    AX.X, ALU.max, negate=True)
                                nc.scalar.activation(
                                    out=e_sb[g][:, hs], in_=logits[g][:, hs],
                                    func=AF.Exp, bias=negmax[g][:, h:h + 1],
                                    scale=1.0, accum_out=zden[g][:, h:h + 1])
                            # combine stats: nfm = -m = min_h(-m_h)
                            nc.vector.tensor_reduce(nfm[g][:], negmax[g][:, 0:2],
                                                    AX.X, ALU.min)
                            # fh = e^{m_h - m} = exp(-((-m_h) - (-m)))
                            nc.vector.tensor_scalar(dmh[g][:], negmax[g][:, 0:2],
                                                    nfm[g][:], None, ALU.subtract)
                            nc.scalar.activation(out=fh[g][:], in_=dmh[g][:],
                                                 func=AF.Exp, bias=0.0, scale=-1.0)
                            nc.vector.tensor_tensor(zf[g][:], zden[g][:, 0:2],
                                                    fh[g][:], ALU.mult)
                            nc.vector.tensor_reduce(zc[g][:], zf[g][:], AX.X, ALU.add)
                            nc.vector.reciprocal(invz[g][:], zc[g][:])
                            nc.vector.tensor_scalar(sc2[g][:], fh[g][:],
                                                    invz[g][:], None, ALU.mult)
                            out_ps = pp_o.tile([GP, N], F32, tag="op")
                            for h in range(2):
                                hs = slice(h * HALF, (h + 1) * HALF)
                                nc.vector.tensor_scalar(
                                    p_sb[g][:, hs], e_sb[g][:, hs],
                                    sc2[g][:, h:h + 1], None, ALU.mult)
                                nc.tensor.matmul(out_ps[:, hs], onesg[:],
                                                 p_sb[g][:, hs], start=True, stop=True)
                            return out_ps

                        def emit_group_finish(g, out_ps):
                            nc.scalar.copy(out_sb[g][:, 0:HALF], out_ps[:, 0:HALF])
                            nc.vector.tensor_copy(out_sb[g][:, HALF:N],
                                                  out_ps[:, HALF:N])
                            nc.sync.dma_start(out=d_out[GP * g:GP * (g + 1), 0:HALF],
                                              in_=out_sb[g][:, 0:HALF])
                            nc.scalar.dma_start(out=d_out[GP * g:GP * (g + 1), HALF:N],
                                                in_=out_sb[g][:, HALF:N])

                        with tc.tile_pool(name="bcast", bufs=3) as bc_pool, \
                             tc.tile_pool(name="scr_act", bufs=6) as scr_act, \
                             tc.tile_pool(name="scr_dve", bufs=6) as scr_dve, \
                             tc.tile_pool(name="scr_gps", bufs=4) as scr_gps:
                            # GPS broadcasts run two pairs ahead of the GPS
                            # B-segs; per-pair consumers are emitted lag-2 so
                            # emission order matches data order on every engine.
                            bcast_of = {}
                            LAG = 2 if GPS_SEGS else 0
                            for it in range(PAIRS + LAG):
                                if it < PAIRS:
                                    bc = bc_pool.tile([D, N], F32, tag="bcast")
                                    bcast_of[it] = bc
                                    src = s_rows[0:1, :] if it == 0 else srow[it][:]
                                    nc.gpsimd.partition_broadcast(bc[:], src)
                                pr = it - LAG
                                if pr < 0:
                                    continue
                                bcast = bcast_of[pr]
                                for gseg in range(2 - GPS_SEGS, 2):
                                    _seg_gps(pr, NCHUNK - 1, gseg)
                                emit_units(pr)
                                emit_pair_brow(pr)
                                if pr == GP - 1:
                                    ops0 = emit_group_tail(0)
                            emit_group_finish(0, ops0)
                            ops1 = emit_group_tail(1)
                            emit_group_finish(1, ops1)
                            if DK_DEBUG:
                                nc.sync.dma_start(out=d_dbg_s[:], in_=s_rows[:])
                                for g in range(GROUPS):
                                    nc.sync.dma_start(
                                        out=d_dbg_m[GP * g:GP * (g + 1), :],
                                        in_=b_rows[g][:])

    nc.finalize()
    return nc


def host_inputs(query, neighbors, gumbel):
    """Per-core input maps. Core c handles pairs [8c, 8c+8)."""
    query = np.asarray(query, np.float32)
    neighbors = np.asarray(neighbors, np.float32)
    gumbel = np.asarray(gumbel, np.float32)

    nbT = np.ascontiguousarray(neighbors.T)
    nbs = np.ascontiguousarray(nbT.sum(1, keepdims=True))
    nb2 = np.sum(neighbors * neighbors, 1)[None, :]
    ident = np.eye(D, dtype=np.float32)

    # logits = (scaling - n) * s + 2*Lo   (see header derivation)
    scaling = (N + 1 - 2 * np.arange(1, K + 1)).astype(np.float32) - float(N)
    lhsg_s = np.zeros((PAIRS, GROUPS * 16 * GP), np.float32)
    lhsg_b = np.zeros((GP, 16 * GP), np.float32)
    onesg = np.zeros((16 * GP, GP), np.float16)
    for q in range(GP):
        for g in range(GROUPS):
            lhsg_s[GP * g + q, 16 * GP * g + 16 * q:16 * GP * g + 16 * q + K] = scaling
        lhsg_b[q, 16 * q:16 * q + K] = 2.0
        onesg[16 * q:16 * q + K, q] = 1.0

    gflat = gumbel.reshape(S * M, N)
    in_maps = []
    for c in range(NCORES):
        m0 = (PAIRS * c) % M
        g8 = np.ascontiguousarray(gflat[PAIRS * c:PAIRS * (c + 1)] - nb2)
        g8_aug = np.concatenate(
            [g8, (-g8.sum(1, keepdims=True) / N).astype(np.float32)], 1)
        in_maps.append({
            "nbT": nbT,
            "nbs": nbs,
            "qT2": np.ascontiguousarray(2.0 * query.T[:, m0:m0 + PAIRS]),
            "gum8": g8_aug,
            "ident": ident,
            "lhsg_s": lhsg_s,
            "lhsg_b": lhsg_b,
            "onesg": onesg,
        })
    return in_maps


_NC_CACHE = {}


def _get_nc():
    if "nc" not in _NC_CACHE:
        _NC_CACHE["nc"] = build_nc()
    return _NC_CACHE["nc"]


def run(query, neighbors, gumbel, trace=False):
    nc = _get_nc()
    in_maps = host_inputs(query, neighbors, gumbel)
    res = run_bass_kernel_spmd(nc, in_maps, list(range(NCORES)), trace=trace)
    outs = np.stack([res.results[c]["out"] for c in range(NCORES)])
    full = outs.reshape(S, M, N).astype(np.float32)
    return full, res


def kernel(query, neighbors, gumbel):
    full, _ = run(query, neighbors, gumbel, trace=False)
    return full


def _numpy_model(query, neighbors, gumbel):
    q = np.asarray(query, np.float32)
    nb = np.asarray(neighbors, np.float32)
    g = np.asarray(gumbel, np.float32).reshape(S * M, N)
    t = 2.0 * q @ nb.T - np.sum(nb * nb, 1)[None, :]
    t = np.concatenate([t, t], 0)
    s = t + g
    s = s - s.mean(1, keepdims=True)
    B = np.abs(s[:, :, None] - s[:, None, :]).sum(2)
    scaling = (N + 1 - 2 * np.arange(1, K + 1)).astype(np.float32)
    l = scaling[None, :, None] * s[:, None, :] - B[:, None, :]
    l = l - l.max(2, keepdims=True)
    e = np.exp(l)
    p = e / e.sum(2, keepdims=True)
    return p.sum(1).reshape(S, M, N)


def _selftest_sim():
    from concourse.bass_interp import CoreSim

    rng = np.random.default_rng(0)
    query = rng.normal(size=(M, D)).astype(np.float32)
    neighbors = rng.normal(size=(N, D)).astype(np.float32)
    u = rng.uniform(1e-6, 1 - 1e-6, size=(S, M, N)).astype(np.float32)
    gumbel = -np.log(-np.log(u)).astype(np.float32)

    nc = _get_nc()
    in_maps = host_inputs(query, neighbors, gumbel)
    sim = CoreSim(nc)
    for k, v in in_maps[0].items():
        sim.tensor(k)[:] = v
    sim.simulate()
    got = np.array(sim.tensor("out"))
    want = _numpy_model(query, neighbors, gumbel).reshape(S * M, N)[:PAIRS]
    err = np.linalg.norm(got - want) / np.linalg.norm(want)
    print("sim rel err:", err)
    print("sim time (model ns):", sim.time)
    assert err < 2e-2, err
    print("SIM PASS")


if __name__ == "__main__":
    if "--sim" in sys.argv:
        _selftest_sim()
